# revision 7
# baseline (speedup 1.0000x reference)
"""Trainium2 kernel for nn_BatchedTorchParametricSolver_81767587381598.

Sharding: pure data parallel over the batch dim (8 batches -> 8 NeuronCores);
the small conv params are replicated (uploaded once and cached device-side).

Per call, the host computes the 8 Gumbel-perturbed memory argsorts (the
neuron compiler rejects sort HLOs, and host argsort is only ~4 ms each),
packs each permutation to 18 bits (u16 lo + 2-bit-packed hi) and streams it
to its core. A hand-written Bass/Tile kernel (one program, SPMD on cores
0-7 via a cached bass2jax/PJRT executable) does the heavy, bandwidth-bound
middle of the pipeline per core:

  unpack perm -> 3x 256x256 f32 images -> 8-ch 3x3 feat conv (VectorE FMAs,
  halo layout) -> TensorE tile transposes -> 1536 indirect-DMA row scatters
  (each places one element's 8-channel feature row at its permuted address
  in a padded HBM table) -> halo readback -> 16-ch 3x3 mem conv (lane-masked
  strided FMAs) -> relu -> block-sum pool -> pooled [4, 64] (256 floats).

Only 8 KB comes back (pooled sums); the result fetch is issued on a
background thread right after dispatch so its ~85 ms axon round-trip
overlaps the upload drain and the host-side work: the Plackett-Luce
suffix-logsumexps, the 65536x256 projection (BLAS), the op argsorts and the
tiered hop penalties, all exact in fp32.

Self-contained: shapes hardcoded; no sibling imports; /opt/trn_rl_repo
provides the concourse (Bass) toolchain preinstalled in this container.
"""
import sys
import threading

import numpy as np

if '/opt/trn_rl_repo' not in sys.path:
    sys.path.insert(0, '/opt/trn_rl_repo')

# ---- static problem structure (hardcoded) ----
OFFS = [0, 65536, 131072, 196608]
N_ELEM = 196608
N_ROWS = 24576
LANE = 8
NUM_OPS = 65536
BATCH = 8
N_CORES = 8
RPP = 192            # addr rows per partition in the device layout
IMG_PAD = 66048      # 256 + 65536 + 256 elements per padded image

_rt = None           # lazily-built device runtime


def _stable_argsort_fast(k):
    """Exact stable argsort at introsort speed: unstable sort, then repair
    exact-tie runs by sorting their indices ascending."""
    p = np.argsort(k)
    ks = k[p]
    eq = np.nonzero(ks[1:] == ks[:-1])[0]
    if eq.size:
        breaks = np.nonzero(np.diff(eq) > 1)[0]
        starts = np.concatenate([eq[:1], eq[breaks + 1]])
        ends = np.concatenate([eq[breaks], eq[-1:]]) + 2
        for s0, e0 in zip(starts, ends):
            p[s0:e0] = np.sort(p[s0:e0])
    return p


def _build_bass(num_devices):
    """Construct + compile the per-core Bass program (see module docstring)."""
    from contextlib import ExitStack
    import concourse.bass as bass
    import concourse.tile as tile
    import concourse.bacc as bacc
    from concourse import mybir
    from concourse.masks import make_identity

    FP32 = mybir.dt.float32
    I32 = mybir.dt.int32
    ALU = mybir.AluOpType
    ACT = mybir.ActivationFunctionType

    nc = bacc.Bacc("TRN2", target_bir_lowering=False, debug=False,
                   enable_asserts=False, num_devices=num_devices)
    lo_d = nc.dram_tensor("perm_lo", [3 * IMG_PAD], mybir.dt.uint16,
                          kind="ExternalInput").ap()
    hi_d = nc.dram_tensor("perm_hi", [3 * IMG_PAD // 4], mybir.dt.uint8,
                          kind="ExternalInput").ap()
    par_d = nc.dram_tensor("params", [1408], FP32, kind="ExternalInput").ap()
    out_ap = nc.dram_tensor("pooled", [4, 64], FP32, kind="ExternalOutput").ap()

    with tile.TileContext(nc) as tc:
        with ExitStack() as ctx:
            io = ctx.enter_context(tc.tile_pool(name="io", bufs=1))
            imgp = ctx.enter_context(tc.tile_pool(name="imgp", bufs=2))
            featp = ctx.enter_context(tc.tile_pool(name="featp", bufs=3))
            bigp = ctx.enter_context(tc.tile_pool(name="bigp", bufs=1))
            psp = ctx.enter_context(tc.tile_pool(name="psp", bufs=2, space="PSUM"))
            psq = ctx.enter_context(tc.tile_pool(name="psq", bufs=2, space="PSUM"))
            accp = ctx.enter_context(tc.tile_pool(name="accp", bufs=2))
            dramp = ctx.enter_context(tc.tile_pool(name="dramp", bufs=1,
                                                   space="DRAM"))

            parb = io.tile([128, 1408], FP32)
            nc.sync.dma_start(parb[:],
                              bass.AP(par_d.tensor, 0, [[0, 128], [1, 1408]]))
            PMW, PMB, MCW, MCB = 0, 216, 240, 1392

            def w_ap(idx):
                return parb[:, idx:idx + 1]

            ident = io.tile([128, 128], FP32)
            make_identity(nc, ident[:])

            X = bigp.tile([128, 12416], FP32, name="X", tag="XM")
            I = bigp.tile([128, 1536], I32)

            # load + unpack the 3 padded images with row halo
            img = [None] * 3
            for m in range(3):
                lo_t = imgp.tile([128, 1024], mybir.dt.uint16, name=f"lo{m}")
                nc.sync.dma_start(lo_t[:], bass.AP(lo_d.tensor, m * IMG_PAD,
                                                   [[512, 128], [1, 1024]]))
                hi_t = imgp.tile([128, 256], mybir.dt.uint8, name=f"hi{m}")
                nc.sync.dma_start(hi_t[:],
                                  bass.AP(hi_d.tensor, m * (IMG_PAD // 4),
                                          [[128, 128], [1, 256]]))
                lo_f = imgp.tile([128, 1024], FP32, name=f"lof{m}")
                nc.vector.tensor_copy(lo_f[:], lo_t[:])
                hi_i = imgp.tile([128, 256], I32, name=f"hii{m}")
                nc.vector.tensor_copy(hi_i[:], hi_t[:])
                hi_f = imgp.tile([128, 1024], FP32, name=f"hif{m}")
                hi_j = imgp.tile([128, 256], I32, name=f"hij{m}")
                hf4 = hi_f[:].rearrange("p (c four) -> p c four", four=4)
                for j in range(4):
                    nc.vector.tensor_scalar(hi_j[:], hi_i[:], 2 * j, 3,
                                            ALU.logical_shift_right,
                                            ALU.bitwise_and)
                    nc.vector.tensor_copy(hf4[:, :, j], hi_j[:])
                im = imgp.tile([128, 1024], FP32, name=f"im{m}")
                nc.vector.scalar_tensor_tensor(im[:], hi_f[:], 65536.0, lo_f[:],
                                               ALU.mult, ALU.add)
                img[m] = im

            # scatter indices: I[p, 512m + 4p' + 2lr + cb] = perm of element
            # e = m*65536 + (2p'+lr)*256 + cb*128 + p
            I4 = I[:].rearrange("p (mm a four) -> p mm a four", mm=3, four=4)
            for m in range(3):
                for lr in range(2):
                    for cb in range(2):
                        ps = psp.tile([128, 128], FP32, space="PSUM")
                        base = 256 + lr * 256 + cb * 128
                        nc.tensor.transpose(ps[:], img[m][:, base:base + 128],
                                            ident[:])
                        nc.vector.tensor_copy(I4[:, m, :, 2 * lr + cb], ps[:])

            # feat conv + transpose into channel-interleaved X
            for m in range(3):
                for ch in range(8):
                    acc = featp.tile([128, 512], FP32, name="facc")
                    nc.scalar.activation(acc[:], img[m][:, 0:512], ACT.Identity,
                                         bias=w_ap(PMB + m * 8 + ch), scale=0.0)
                    for dr in range(3):
                        for dl in range(3):
                            w = w_ap(PMW + ((m * 8 + ch) * 3 + dr) * 3 + dl)
                            oc0, ic0 = (1, 0) if dl == 0 else (0, dl - 1)
                            nw = 255 if dl != 1 else 256
                            out_sl = acc[:].rearrange(
                                "p (two c) -> p two c", two=2)[:, :, oc0:oc0 + nw]
                            in_sl = img[m][:].rearrange(
                                "p (four c) -> p four c",
                                four=4)[:, dr:dr + 2, ic0:ic0 + nw]
                            nc.vector.scalar_tensor_tensor(
                                out_sl, in_sl, w, out_sl, ALU.mult, ALU.add)
                    nc.scalar.activation(acc[:], acc[:], ACT.Relu)
                    X4 = X[:, 0:12288].rearrange("p (mm pp f) -> p mm pp f",
                                                 mm=3, f=32)
                    for lr in range(2):
                        for cb in range(2):
                            ps = psp.tile([128, 128], FP32, space="PSUM")
                            nc.tensor.transpose(
                                ps[:], acc[:, lr * 256 + cb * 128:
                                           lr * 256 + cb * 128 + 128], ident[:])
                            nc.vector.tensor_copy(
                                X4[:, m, :, 16 * lr + 8 * cb + ch], ps[:])

            # +8 accounts for the zero-padded table head row
            nc.vector.tensor_scalar(I[:], I[:], 8, None, ALU.add)

            zpad = io.tile([1, 64], FP32)
            nc.gpsimd.memset(zpad[:], 0.0)

            # per-element indirect scatters: one 8ch row per partition/inst
            T = dramp.tile([N_ELEM + 16, 8], FP32, name="T")
            nc.sync.dma_start(bass.AP(T.tensor, 0, [[64, 1], [1, 64]]), zpad[:])
            nc.sync.dma_start(bass.AP(T.tensor, (N_ELEM + 8) * 8,
                                      [[64, 1], [1, 64]]), zpad[:])
            for k in range(1536):
                nc.gpsimd.indirect_dma_start(
                    out=T[:, :],
                    out_offset=bass.IndirectOffsetOnAxis(ap=I[:, k:k + 1],
                                                         axis=0),
                    in_=X[:, 8 * k:8 * k + 8],
                    in_offset=None)

            # halo readback: partition p covers addr rows 192p-1 .. 192p+193
            ms_ov = bigp.tile([128, 12416], FP32, name="ms_ov", tag="XM")
            nc.sync.dma_start(ms_ov[:],
                              bass.AP(T.tensor, 0, [[12288, 128], [1, 12416]]))

            blockmask = io.tile([128, 4], FP32)
            nc.gpsimd.memset(blockmask[:], 0.0)
            for bi in range(4):
                nc.gpsimd.memset(blockmask[bi * 32:(bi + 1) * 32, bi:bi + 1], 1.0)

            pooled = io.tile([4, 64], FP32)
            ms_p = ms_ov[:].ap[0]
            for co in range(16):
                acc = accp.tile([128, 1536], FP32, name="macc")
                acc3 = acc[:].rearrange("p (r l) -> p r l", l=LANE)
                nc.scalar.activation(acc[:], ms_ov[:, 0:1536], ACT.Identity,
                                     bias=w_ap(MCB + co), scale=0.0)
                for ci in range(8):
                    for dr in range(3):
                        for dl in range(3):
                            w = w_ap(MCW + ((co * 8 + ci) * 3 + dr) * 3 + dl)
                            ol0, il0 = (1, 0) if dl == 0 else (0, dl - 1)
                            nl = 7 if dl != 1 else 8
                            in_sl = bass.AP(ms_ov.tensor,
                                            dr * 64 + il0 * 8 + ci,
                                            [list(ms_p), [64, RPP], [8, nl]])
                            nc.vector.scalar_tensor_tensor(
                                acc3[:, :, ol0:ol0 + nl], in_sl,
                                w, acc3[:, :, ol0:ol0 + nl], ALU.mult, ALU.add)
                nc.scalar.activation(acc[:], acc[:], ACT.Relu)
                red = accp.tile([128, 4], FP32, name="red")
                for bj in range(4):
                    nc.vector.tensor_reduce(red[:, bj:bj + 1],
                                            acc3[:, :, 2 * bj:2 * bj + 2],
                                            mybir.AxisListType.XY, ALU.add)
                ps = psq.tile([4, 4], FP32, space="PSUM")
                nc.tensor.matmul(ps[:], blockmask[:], red[:], start=True,
                                 stop=True)
                nc.vector.tensor_copy(pooled[:, co * 4:co * 4 + 4], ps[:])

            nc.sync.dma_start(out_ap, pooled[:])
    nc.compile()
    return nc


class _Runtime:
    """Cached jitted SPMD executable + device-resident params."""

    def __init__(self):
        import jax
        from jax.sharding import Mesh, PartitionSpec, NamedSharding
        from jax.experimental.shard_map import shard_map
        from concourse.bass2jax import (install_neuronx_cc_hook, _bass_exec_p,
                                        partition_id_tensor)
        from concourse import mybir
        self.jax = jax
        self.devices = jax.devices()[:N_CORES]
        nc = _build_bass(N_CORES)
        install_neuronx_cc_hook()

        in_names, out_names, out_avals, zero_outs = [], [], [], []
        for alloc in nc.m.functions[0].allocations:
            if not isinstance(alloc, mybir.MemoryLocationSet):
                continue
            name = alloc.memorylocations[0].name
            if alloc.kind == "ExternalInput":
                if name != "partition_id":
                    in_names.append(name)
            elif alloc.kind == "ExternalOutput":
                out_names.append(name)
                shape = tuple(alloc.tensor_shape)
                dtype = mybir.dt.np(alloc.dtype)
                out_avals.append(jax.core.ShapedArray(shape, dtype))
                zero_outs.append(np.zeros(shape, dtype))
        all_names = in_names + out_names + ["partition_id"]
        self.in_names = in_names
        self.zero_outs = zero_outs
        n_params, n_outs = len(in_names), len(out_names)

        def _body(*args):
            ops = list(args) + [partition_id_tensor()]
            outs = _bass_exec_p.bind(
                *ops, out_avals=tuple(out_avals), in_names=tuple(all_names),
                out_names=tuple(out_names), lowering_input_output_aliases=(),
                sim_require_finite=True, sim_require_nnan=True, nc=nc)
            return tuple(outs)

        self.mesh = Mesh(np.asarray(self.devices), ("core",))
        self.sh = NamedSharding(self.mesh, PartitionSpec("core"))
        self.sharded = jax.jit(
            shard_map(_body, mesh=self.mesh,
                      in_specs=(PartitionSpec("core"),) * (n_params + n_outs),
                      out_specs=(PartitionSpec("core"),) * n_outs),
            donate_argnums=tuple(range(n_params, n_params + n_outs)),
            keep_unused=True)
        self._params_key = None
        self._params_g = None

    def params_global(self, params_np):
        key = params_np.tobytes()
        if self._params_key != key:
            parts = [self.jax.device_put(params_np, d) for d in self.devices]
            self._params_g = self.jax.make_array_from_single_device_arrays(
                (N_CORES * 1408,), self.sh, parts)
            self._params_key = key
        return self._params_g

    def assemble(self, parts):
        shape = (N_CORES * parts[0].shape[0],) + tuple(parts[0].shape[1:])
        return self.jax.make_array_from_single_device_arrays(
            shape, self.sh, parts)

    def dispatch(self, lo_parts, hi_parts, params_np):
        lo_g = self.assemble(lo_parts)
        hi_g = self.assemble(hi_parts)
        par_g = self.params_global(params_np)
        args = {"perm_lo": lo_g, "perm_hi": hi_g, "params": par_g}
        zeros = [np.zeros((N_CORES * z.shape[0],) + z.shape[1:], z.dtype)
                 for z in self.zero_outs]
        out = self.sharded(*[args[n] for n in self.in_names], *zeros)
        return out[0]


def _pack_perm(perm):
    """perm int32[196608] -> padded u16 lo + 2-bit-packed u8 hi."""
    padded = np.zeros(3 * IMG_PAD, np.int32)
    pv = padded.reshape(3, IMG_PAD)
    pv[:, 256:256 + 65536] = perm.reshape(3, 65536)
    lo = (padded & 0xFFFF).astype(np.uint16)
    hi4 = (padded >> 16).astype(np.uint8).reshape(-1, 4)
    hi = (hi4[:, 0] | (hi4[:, 1] << 2) | (hi4[:, 2] << 4)
          | (hi4[:, 3] << 6)).astype(np.uint8)
    return lo, hi


def _tier_small(h):
    return np.where(h <= 2, 1.0, np.where(h <= 4, 1.5,
           np.where(h <= 8, 2.0, 3.0))).astype(np.float32)


def _staged(v):
    """sum(fwd*tier(fwd) + bwd^2*tier(bwd)) per row; values are integer
    address deltas, almost all in tier 5 -- take the tier-5 base (BLAS dot
    for the squared half) and correct the few small-magnitude entries."""
    fwd = np.maximum(v, 0)
    bwd = fwd - v
    out = np.float32(5.0) * (fwd.sum(axis=-1, dtype=np.float32)
                             + np.einsum('ij,ij->i', bwd, bwd))
    bi, ei = np.nonzero((np.abs(v) < 17) & (v != 0))
    if bi.size:
        s = v[bi, ei]
        mag = np.abs(s)
        corr = (np.float32(5.0) - _tier_small(mag)) * np.where(s > 0, mag,
                                                               mag * mag)
        out -= np.bincount(bi, weights=corr, minlength=v.shape[0]
                           ).astype(np.float32)
    return out.astype(np.float32)


def kernel(mem_logits, gumbel_mem, gumbel_op, pm_conv_w, pm_conv_b,
           mem_conv_w, mem_conv_b, proj_w, proj_b):
    """Full (unsharded) inputs -> full (4, BATCH) float32 output."""
    global _rt
    import jax
    if _rt is None:
        _rt = _Runtime()

    mem_logits = np.asarray(mem_logits, dtype=np.float32)
    gumbel_mem = np.asarray(gumbel_mem, dtype=np.float32)
    gumbel_op = np.asarray(gumbel_op, dtype=np.float32)
    proj_w = np.asarray(proj_w, dtype=np.float32)
    proj_b = np.asarray(proj_b, dtype=np.float32)
    params = np.concatenate([
        np.asarray(pm_conv_w, np.float32).reshape(-1),
        np.asarray(pm_conv_b, np.float32).reshape(-1),
        np.asarray(mem_conv_w, np.float32).reshape(-1),
        np.asarray(mem_conv_b, np.float32).reshape(-1)]).astype(np.float32)

    # ---- memory permutations (host argsort), streamed per batch ----
    # The batched two-list device_put path dispatches in ~1 ms; the axon
    # client feeds the link from this thread's slack. A background thread
    # only handles the final result fetch, so its ~85 ms round-trip
    # overlaps the host math below.
    keys = mem_logits + gumbel_mem
    perm = np.empty((BATCH, N_ELEM), np.int32)
    lo_parts, hi_parts = [], []
    for b in range(BATCH):
        perm[b] = _stable_argsort_fast(keys[b])
        lo, hi = _pack_perm(perm[b])
        d = _rt.devices[b]
        lo_b, hi_b = jax.device_put([lo, hi], [d, d])
        lo_parts.append(lo_b)
        hi_parts.append(hi_b)

    fut = _rt.dispatch(lo_parts, hi_parts, params)
    result = {}

    def _fetch():
        result["pooled"] = np.asarray(fut)

    th = threading.Thread(target=_fetch)
    th.start()

    # ---- host work overlapped with the device leg ----
    mem_lp = np.empty((BATCH,), np.float32)
    for b in range(BATCH):
        s = mem_logits[b][perm[b]]
        m = s[-1]
        e = np.exp(s - m, dtype=np.float32)
        suf = np.cumsum(e[::-1], dtype=np.float32)[::-1]
        mem_lp[b] = (s.sum(dtype=np.float32)
                     - (np.log(suf).sum(dtype=np.float32)
                        + np.float32(N_ELEM) * m))
    A = perm[:, OFFS[0]:OFFS[1]].astype(np.float32)
    Bc = perm[:, OFFS[1]:OFFS[2]].astype(np.float32)
    Cc = perm[:, OFFS[2]:OFFS[3]].astype(np.float32)
    intra_pen = _staged(np.concatenate([Bc - A, Cc - Bc], axis=1))

    th.join()
    dev = result["pooled"].reshape(BATCH, 4, 16, 4)       # [b, bi, co, bj]
    pooled = (dev.transpose(0, 2, 1, 3).reshape(BATCH, 256)
              * np.float32(1.0 / 12288.0))

    # ---- projection + op path (host) ----
    opl = (proj_w @ pooled.T).T
    opl += proj_b
    opl = opl.astype(np.float32)
    op_lp = np.empty((BATCH,), np.float32)
    inter_d = np.empty((BATCH, NUM_OPS - 1), np.float32)
    for b in range(BATCH):
        o = _stable_argsort_fast(opl[b] + gumbel_op[b])
        so = opl[b][o]
        m = so[-1]
        e = np.exp(so - m, dtype=np.float32)
        suf = np.cumsum(e[::-1], dtype=np.float32)[::-1]
        op_lp[b] = (so.sum(dtype=np.float32)
                    - (np.log(suf).sum(dtype=np.float32)
                       + np.float32(NUM_OPS) * m))
        Ao = A[b][o]
        Co = Cc[b][o]
        inter_d[b] = Ao[1:] - Co[:-1]
    inter_pen = _staged(inter_d)

    out = np.stack([inter_pen, intra_pen, op_lp, mem_lp]).astype(np.float32)
    return np.ascontiguousarray(out)


# revision 8
# speedup vs baseline: 1.1606x; 1.1606x over previous
"""Trainium2 kernel for nn_BatchedTorchParametricSolver_81767587381598.

Sharding: pure data parallel over the batch dim (8 batches -> 8 NeuronCores);
the small conv params are replicated (uploaded once and cached device-side).

Per call, the host computes the 8 Gumbel-perturbed memory argsorts (the
neuron compiler rejects sort HLOs, and host argsort is only ~4 ms each),
packs each permutation to 18 bits (u16 lo + 2-bit-packed hi) and streams it
to its core. A hand-written Bass/Tile kernel (one program, SPMD on cores
0-7 via a cached bass2jax/PJRT executable) does the heavy, bandwidth-bound
middle of the pipeline per core:

  unpack perm -> 3x 256x256 f32 images -> 8-ch 3x3 feat conv (VectorE FMAs,
  halo layout) -> TensorE tile transposes -> 1536 indirect-DMA row scatters
  (each places one element's 8-channel feature row at its permuted address
  in a padded HBM table) -> halo readback -> 16-ch 3x3 mem conv (lane-masked
  strided FMAs) -> relu -> block-sum pool -> pooled [4, 64] (256 floats).

Only 8 KB comes back (pooled sums); the result fetch is issued on a
background thread right after dispatch so its ~85 ms axon round-trip
overlaps the upload drain and the host-side work: the Plackett-Luce
suffix-logsumexps, the 65536x256 projection (BLAS), the op argsorts and the
tiered hop penalties, all exact in fp32.

Self-contained: shapes hardcoded; no sibling imports; /opt/trn_rl_repo
provides the concourse (Bass) toolchain preinstalled in this container.
"""
import sys
import threading

import numpy as np

if '/opt/trn_rl_repo' not in sys.path:
    sys.path.insert(0, '/opt/trn_rl_repo')

# ---- static problem structure (hardcoded) ----
OFFS = [0, 65536, 131072, 196608]
N_ELEM = 196608
N_ROWS = 24576
LANE = 8
NUM_OPS = 65536
BATCH = 8
N_CORES = 8
RPP = 192            # addr rows per partition in the device layout
IMG_PAD = 66048      # 256 + 65536 + 256 elements per padded image

_rt = None           # lazily-built device runtime


def _stable_argsort_fast(k):
    """Exact stable argsort at introsort speed: unstable sort, then repair
    exact-tie runs by sorting their indices ascending."""
    p = np.argsort(k)
    ks = k[p]
    eq = np.nonzero(ks[1:] == ks[:-1])[0]
    if eq.size:
        breaks = np.nonzero(np.diff(eq) > 1)[0]
        starts = np.concatenate([eq[:1], eq[breaks + 1]])
        ends = np.concatenate([eq[breaks], eq[-1:]]) + 2
        for s0, e0 in zip(starts, ends):
            p[s0:e0] = np.sort(p[s0:e0])
    return p


def _build_bass(num_devices):
    """Construct + compile the per-core Bass program (see module docstring)."""
    from contextlib import ExitStack
    import concourse.bass as bass
    import concourse.tile as tile
    import concourse.bacc as bacc
    from concourse import mybir
    from concourse.masks import make_identity

    FP32 = mybir.dt.float32
    I32 = mybir.dt.int32
    ALU = mybir.AluOpType
    ACT = mybir.ActivationFunctionType

    nc = bacc.Bacc("TRN2", target_bir_lowering=False, debug=False,
                   enable_asserts=False, num_devices=num_devices,
                   num_swdge_queues=4)
    lo_d = nc.dram_tensor("perm_lo", [3 * IMG_PAD], mybir.dt.uint16,
                          kind="ExternalInput").ap()
    hi_d = nc.dram_tensor("perm_hi", [3 * IMG_PAD // 4], mybir.dt.uint8,
                          kind="ExternalInput").ap()
    par_d = nc.dram_tensor("params", [1408], FP32, kind="ExternalInput").ap()
    out_ap = nc.dram_tensor("pooled", [4, 64], FP32, kind="ExternalOutput").ap()

    with tile.TileContext(nc) as tc:
        with ExitStack() as ctx:
            io = ctx.enter_context(tc.tile_pool(name="io", bufs=1))
            imgp = ctx.enter_context(tc.tile_pool(name="imgp", bufs=2))
            featp = ctx.enter_context(tc.tile_pool(name="featp", bufs=3))
            bigp = ctx.enter_context(tc.tile_pool(name="bigp", bufs=1))
            psp = ctx.enter_context(tc.tile_pool(name="psp", bufs=2, space="PSUM"))
            psq = ctx.enter_context(tc.tile_pool(name="psq", bufs=2, space="PSUM"))
            accp = ctx.enter_context(tc.tile_pool(name="accp", bufs=2))
            dramp = ctx.enter_context(tc.tile_pool(name="dramp", bufs=1,
                                                   space="DRAM"))

            parb = io.tile([128, 1408], FP32)
            nc.sync.dma_start(parb[:],
                              bass.AP(par_d.tensor, 0, [[0, 128], [1, 1408]]))
            PMW, PMB, MCW, MCB = 0, 216, 240, 1392

            def w_ap(idx):
                return parb[:, idx:idx + 1]

            ident = io.tile([128, 128], FP32)
            make_identity(nc, ident[:])

            X = bigp.tile([128, 12416], FP32, name="X", tag="XM")
            I = bigp.tile([128, 1536], I32)

            # load + unpack the 3 padded images with row halo
            img = [None] * 3
            for m in range(3):
                lo_t = imgp.tile([128, 1024], mybir.dt.uint16, name=f"lo{m}")
                nc.sync.dma_start(lo_t[:], bass.AP(lo_d.tensor, m * IMG_PAD,
                                                   [[512, 128], [1, 1024]]))
                hi_t = imgp.tile([128, 256], mybir.dt.uint8, name=f"hi{m}")
                nc.sync.dma_start(hi_t[:],
                                  bass.AP(hi_d.tensor, m * (IMG_PAD // 4),
                                          [[128, 128], [1, 256]]))
                lo_f = imgp.tile([128, 1024], FP32, name=f"lof{m}")
                nc.vector.tensor_copy(lo_f[:], lo_t[:])
                hi_i = imgp.tile([128, 256], I32, name=f"hii{m}")
                nc.vector.tensor_copy(hi_i[:], hi_t[:])
                hi_f = imgp.tile([128, 1024], FP32, name=f"hif{m}")
                hi_j = imgp.tile([128, 256], I32, name=f"hij{m}")
                hf4 = hi_f[:].rearrange("p (c four) -> p c four", four=4)
                for j in range(4):
                    nc.vector.tensor_scalar(hi_j[:], hi_i[:], 2 * j, 3,
                                            ALU.logical_shift_right,
                                            ALU.bitwise_and)
                    nc.vector.tensor_copy(hf4[:, :, j], hi_j[:])
                im = imgp.tile([128, 1024], FP32, name=f"im{m}")
                nc.vector.scalar_tensor_tensor(im[:], hi_f[:], 65536.0, lo_f[:],
                                               ALU.mult, ALU.add)
                img[m] = im

            # scatter indices: I[p, 512m + 4p' + 2lr + cb] = perm of element
            # e = m*65536 + (2p'+lr)*256 + cb*128 + p
            I4 = I[:].rearrange("p (mm a four) -> p mm a four", mm=3, four=4)
            for m in range(3):
                for lr in range(2):
                    for cb in range(2):
                        ps = psp.tile([128, 128], FP32, space="PSUM")
                        base = 256 + lr * 256 + cb * 128
                        nc.tensor.transpose(ps[:], img[m][:, base:base + 128],
                                            ident[:])
                        nc.vector.tensor_copy(I4[:, m, :, 2 * lr + cb], ps[:])

            # feat conv + transpose into channel-interleaved X
            for m in range(3):
                for ch in range(8):
                    acc = featp.tile([128, 512], FP32, name="facc")
                    nc.scalar.activation(acc[:], img[m][:, 0:512], ACT.Identity,
                                         bias=w_ap(PMB + m * 8 + ch), scale=0.0)
                    for dr in range(3):
                        for dl in range(3):
                            w = w_ap(PMW + ((m * 8 + ch) * 3 + dr) * 3 + dl)
                            oc0, ic0 = (1, 0) if dl == 0 else (0, dl - 1)
                            nw = 255 if dl != 1 else 256
                            out_sl = acc[:].rearrange(
                                "p (two c) -> p two c", two=2)[:, :, oc0:oc0 + nw]
                            in_sl = img[m][:].rearrange(
                                "p (four c) -> p four c",
                                four=4)[:, dr:dr + 2, ic0:ic0 + nw]
                            nc.vector.scalar_tensor_tensor(
                                out_sl, in_sl, w, out_sl, ALU.mult, ALU.add)
                    nc.scalar.activation(acc[:], acc[:], ACT.Relu)
                    X4 = X[:, 0:12288].rearrange("p (mm pp f) -> p mm pp f",
                                                 mm=3, f=32)
                    for lr in range(2):
                        for cb in range(2):
                            ps = psp.tile([128, 128], FP32, space="PSUM")
                            nc.tensor.transpose(
                                ps[:], acc[:, lr * 256 + cb * 128:
                                           lr * 256 + cb * 128 + 128], ident[:])
                            nc.vector.tensor_copy(
                                X4[:, m, :, 16 * lr + 8 * cb + ch], ps[:])

            # +8 accounts for the zero-padded table head row
            nc.vector.tensor_scalar(I[:], I[:], 8, None, ALU.add)

            zpad = io.tile([1, 64], FP32)
            nc.gpsimd.memset(zpad[:], 0.0)

            # per-element indirect scatters: one 8ch row per partition/inst
            T = dramp.tile([N_ELEM + 16, 8], FP32, name="T")
            nc.sync.dma_start(bass.AP(T.tensor, 0, [[64, 1], [1, 64]]), zpad[:])
            nc.sync.dma_start(bass.AP(T.tensor, (N_ELEM + 8) * 8,
                                      [[64, 1], [1, 64]]), zpad[:])
            qnames = ["qPoolDynamic", "qPoolDynamic1", "qPoolDynamic2",
                      "qPoolDynamic3"]
            for k in range(1536):
                inst = nc.gpsimd.indirect_dma_start(
                    out=T[:, :],
                    out_offset=bass.IndirectOffsetOnAxis(ap=I[:, k:k + 1],
                                                         axis=0),
                    in_=X[:, 8 * k:8 * k + 8],
                    in_offset=None)
                inst.queue = qnames[k % 4]

            # halo readback: partition p covers addr rows 192p-1 .. 192p+193
            ms_ov = bigp.tile([128, 12416], FP32, name="ms_ov", tag="XM")
            nc.sync.dma_start(ms_ov[:],
                              bass.AP(T.tensor, 0, [[12288, 128], [1, 12416]]))

            blockmask = io.tile([128, 4], FP32)
            nc.gpsimd.memset(blockmask[:], 0.0)
            for bi in range(4):
                nc.gpsimd.memset(blockmask[bi * 32:(bi + 1) * 32, bi:bi + 1], 1.0)

            pooled = io.tile([4, 64], FP32)
            ms_p = ms_ov[:].ap[0]
            for co in range(16):
                acc = accp.tile([128, 1536], FP32, name="macc")
                acc3 = acc[:].rearrange("p (r l) -> p r l", l=LANE)
                nc.scalar.activation(acc[:], ms_ov[:, 0:1536], ACT.Identity,
                                     bias=w_ap(MCB + co), scale=0.0)
                for ci in range(8):
                    for dr in range(3):
                        for dl in range(3):
                            w = w_ap(MCW + ((co * 8 + ci) * 3 + dr) * 3 + dl)
                            ol0, il0 = (1, 0) if dl == 0 else (0, dl - 1)
                            nl = 7 if dl != 1 else 8
                            in_sl = bass.AP(ms_ov.tensor,
                                            dr * 64 + il0 * 8 + ci,
                                            [list(ms_p), [64, RPP], [8, nl]])
                            nc.vector.scalar_tensor_tensor(
                                acc3[:, :, ol0:ol0 + nl], in_sl,
                                w, acc3[:, :, ol0:ol0 + nl], ALU.mult, ALU.add)
                nc.scalar.activation(acc[:], acc[:], ACT.Relu)
                red = accp.tile([128, 4], FP32, name="red")
                for bj in range(4):
                    nc.vector.tensor_reduce(red[:, bj:bj + 1],
                                            acc3[:, :, 2 * bj:2 * bj + 2],
                                            mybir.AxisListType.XY, ALU.add)
                ps = psq.tile([4, 4], FP32, space="PSUM")
                nc.tensor.matmul(ps[:], blockmask[:], red[:], start=True,
                                 stop=True)
                nc.vector.tensor_copy(pooled[:, co * 4:co * 4 + 4], ps[:])

            nc.sync.dma_start(out_ap, pooled[:])
    nc.compile()
    return nc


class _Runtime:
    """Cached jitted SPMD executable + device-resident params."""

    def __init__(self):
        import jax
        from jax.sharding import Mesh, PartitionSpec, NamedSharding
        from jax.experimental.shard_map import shard_map
        from concourse.bass2jax import (install_neuronx_cc_hook, _bass_exec_p,
                                        partition_id_tensor)
        from concourse import mybir
        self.jax = jax
        self.devices = jax.devices()[:N_CORES]
        nc = _build_bass(N_CORES)
        install_neuronx_cc_hook()

        in_names, out_names, out_avals, zero_outs = [], [], [], []
        for alloc in nc.m.functions[0].allocations:
            if not isinstance(alloc, mybir.MemoryLocationSet):
                continue
            name = alloc.memorylocations[0].name
            if alloc.kind == "ExternalInput":
                if name != "partition_id":
                    in_names.append(name)
            elif alloc.kind == "ExternalOutput":
                out_names.append(name)
                shape = tuple(alloc.tensor_shape)
                dtype = mybir.dt.np(alloc.dtype)
                out_avals.append(jax.core.ShapedArray(shape, dtype))
                zero_outs.append(np.zeros(shape, dtype))
        all_names = in_names + out_names + ["partition_id"]
        self.in_names = in_names
        self.zero_outs = zero_outs
        n_params, n_outs = len(in_names), len(out_names)

        def _body(*args):
            ops = list(args) + [partition_id_tensor()]
            outs = _bass_exec_p.bind(
                *ops, out_avals=tuple(out_avals), in_names=tuple(all_names),
                out_names=tuple(out_names), lowering_input_output_aliases=(),
                sim_require_finite=True, sim_require_nnan=True, nc=nc)
            return tuple(outs)

        self.mesh = Mesh(np.asarray(self.devices), ("core",))
        self.sh = NamedSharding(self.mesh, PartitionSpec("core"))
        self.sharded = jax.jit(
            shard_map(_body, mesh=self.mesh,
                      in_specs=(PartitionSpec("core"),) * (n_params + n_outs),
                      out_specs=(PartitionSpec("core"),) * n_outs),
            donate_argnums=tuple(range(n_params, n_params + n_outs)),
            keep_unused=True)
        self._params_key = None
        self._params_g = None

    def params_global(self, params_np):
        key = params_np.tobytes()
        if self._params_key != key:
            parts = [self.jax.device_put(params_np, d) for d in self.devices]
            self._params_g = self.jax.make_array_from_single_device_arrays(
                (N_CORES * 1408,), self.sh, parts)
            self._params_key = key
        return self._params_g

    def assemble(self, parts):
        shape = (N_CORES * parts[0].shape[0],) + tuple(parts[0].shape[1:])
        return self.jax.make_array_from_single_device_arrays(
            shape, self.sh, parts)

    def dispatch(self, lo_parts, hi_parts, params_np):
        lo_g = self.assemble(lo_parts)
        hi_g = self.assemble(hi_parts)
        par_g = self.params_global(params_np)
        args = {"perm_lo": lo_g, "perm_hi": hi_g, "params": par_g}
        zeros = [np.zeros((N_CORES * z.shape[0],) + z.shape[1:], z.dtype)
                 for z in self.zero_outs]
        out = self.sharded(*[args[n] for n in self.in_names], *zeros)
        return out[0]


def _pack_perm(perm):
    """perm int32[196608] -> padded u16 lo + 2-bit-packed u8 hi."""
    padded = np.zeros(3 * IMG_PAD, np.int32)
    pv = padded.reshape(3, IMG_PAD)
    pv[:, 256:256 + 65536] = perm.reshape(3, 65536)
    lo = (padded & 0xFFFF).astype(np.uint16)
    hi4 = (padded >> 16).astype(np.uint8).reshape(-1, 4)
    hi = (hi4[:, 0] | (hi4[:, 1] << 2) | (hi4[:, 2] << 4)
          | (hi4[:, 3] << 6)).astype(np.uint8)
    return lo, hi


def _tier_small(h):
    return np.where(h <= 2, 1.0, np.where(h <= 4, 1.5,
           np.where(h <= 8, 2.0, 3.0))).astype(np.float32)


def _staged(v):
    """sum(fwd*tier(fwd) + bwd^2*tier(bwd)) per row; values are integer
    address deltas, almost all in tier 5 -- take the tier-5 base (BLAS dot
    for the squared half) and correct the few small-magnitude entries."""
    fwd = np.maximum(v, 0)
    bwd = fwd - v
    out = np.float32(5.0) * (fwd.sum(axis=-1, dtype=np.float32)
                             + np.einsum('ij,ij->i', bwd, bwd))
    bi, ei = np.nonzero((np.abs(v) < 17) & (v != 0))
    if bi.size:
        s = v[bi, ei]
        mag = np.abs(s)
        corr = (np.float32(5.0) - _tier_small(mag)) * np.where(s > 0, mag,
                                                               mag * mag)
        out -= np.bincount(bi, weights=corr, minlength=v.shape[0]
                           ).astype(np.float32)
    return out.astype(np.float32)


def kernel(mem_logits, gumbel_mem, gumbel_op, pm_conv_w, pm_conv_b,
           mem_conv_w, mem_conv_b, proj_w, proj_b):
    """Full (unsharded) inputs -> full (4, BATCH) float32 output."""
    global _rt
    import jax
    if _rt is None:
        _rt = _Runtime()

    mem_logits = np.asarray(mem_logits, dtype=np.float32)
    gumbel_mem = np.asarray(gumbel_mem, dtype=np.float32)
    gumbel_op = np.asarray(gumbel_op, dtype=np.float32)
    proj_w = np.asarray(proj_w, dtype=np.float32)
    proj_b = np.asarray(proj_b, dtype=np.float32)
    params = np.concatenate([
        np.asarray(pm_conv_w, np.float32).reshape(-1),
        np.asarray(pm_conv_b, np.float32).reshape(-1),
        np.asarray(mem_conv_w, np.float32).reshape(-1),
        np.asarray(mem_conv_b, np.float32).reshape(-1)]).astype(np.float32)

    # ---- memory permutations (host argsort), streamed per batch ----
    # The batched two-list device_put path dispatches in ~1 ms; the axon
    # client feeds the link from this thread's slack. A background thread
    # only handles the final result fetch, so its ~85 ms round-trip
    # overlaps the host math below.
    keys = mem_logits + gumbel_mem
    perm = np.empty((BATCH, N_ELEM), np.int32)
    lo_parts, hi_parts = [], []
    for b in range(BATCH):
        perm[b] = _stable_argsort_fast(keys[b])
        lo, hi = _pack_perm(perm[b])
        d = _rt.devices[b]
        lo_b, hi_b = jax.device_put([lo, hi], [d, d])
        lo_parts.append(lo_b)
        hi_parts.append(hi_b)

    fut = _rt.dispatch(lo_parts, hi_parts, params)
    result = {}

    def _fetch():
        result["pooled"] = np.asarray(fut)

    th = threading.Thread(target=_fetch)
    th.start()

    # ---- host work overlapped with the device leg ----
    mem_lp = np.empty((BATCH,), np.float32)
    for b in range(BATCH):
        s = mem_logits[b][perm[b]]
        m = s[-1]
        e = np.exp(s - m, dtype=np.float32)
        suf = np.cumsum(e[::-1], dtype=np.float32)[::-1]
        mem_lp[b] = (s.sum(dtype=np.float32)
                     - (np.log(suf).sum(dtype=np.float32)
                        + np.float32(N_ELEM) * m))
    A = perm[:, OFFS[0]:OFFS[1]].astype(np.float32)
    Bc = perm[:, OFFS[1]:OFFS[2]].astype(np.float32)
    Cc = perm[:, OFFS[2]:OFFS[3]].astype(np.float32)
    intra_pen = _staged(np.concatenate([Bc - A, Cc - Bc], axis=1))

    th.join()
    dev = result["pooled"].reshape(BATCH, 4, 16, 4)       # [b, bi, co, bj]
    pooled = (dev.transpose(0, 2, 1, 3).reshape(BATCH, 256)
              * np.float32(1.0 / 12288.0))

    # ---- projection + op path (host) ----
    opl = (proj_w @ pooled.T).T
    opl += proj_b
    opl = opl.astype(np.float32)
    op_lp = np.empty((BATCH,), np.float32)
    inter_d = np.empty((BATCH, NUM_OPS - 1), np.float32)
    for b in range(BATCH):
        o = _stable_argsort_fast(opl[b] + gumbel_op[b])
        so = opl[b][o]
        m = so[-1]
        e = np.exp(so - m, dtype=np.float32)
        suf = np.cumsum(e[::-1], dtype=np.float32)[::-1]
        op_lp[b] = (so.sum(dtype=np.float32)
                    - (np.log(suf).sum(dtype=np.float32)
                       + np.float32(NUM_OPS) * m))
        Ao = A[b][o]
        Co = Cc[b][o]
        inter_d[b] = Ao[1:] - Co[:-1]
    inter_pen = _staged(inter_d)

    out = np.stack([inter_pen, intra_pen, op_lp, mem_lp]).astype(np.float32)
    return np.ascontiguousarray(out)


# revision 9
# speedup vs baseline: 1.1840x; 1.0202x over previous
"""Trainium2 kernel for nn_BatchedTorchParametricSolver_81767587381598.

Sharding: pure data parallel over the batch dim (8 batches -> 8 NeuronCores);
the small conv params are replicated (uploaded once and cached device-side).

Per call, the host computes the 8 Gumbel-perturbed memory argsorts (the
neuron compiler rejects sort HLOs, and host argsort is only ~4 ms each),
packs each permutation to 18 bits (u16 lo + 2-bit-packed hi) and streams it
to its core. A hand-written Bass/Tile kernel (one program, SPMD on cores
0-7 via a cached bass2jax/PJRT executable) does the heavy, bandwidth-bound
middle of the pipeline per core:

  unpack perm -> 3x 256x256 f32 images -> 8-ch 3x3 feat conv (VectorE FMAs,
  halo layout) -> TensorE tile transposes -> 1536 indirect-DMA row scatters
  (each places one element's 8-channel feature row at its permuted address
  in a padded HBM table) -> halo readback -> 16-ch 3x3 mem conv (lane-masked
  strided FMAs) -> relu -> block-sum pool -> pooled [4, 64] (256 floats).

Only 8 KB comes back (pooled sums); the result fetch is issued on a
background thread right after dispatch so its ~85 ms axon round-trip
overlaps the upload drain and the host-side work: the Plackett-Luce
suffix-logsumexps, the 65536x256 projection (BLAS), the op argsorts and the
tiered hop penalties, all exact in fp32.

Self-contained: shapes hardcoded; no sibling imports; /opt/trn_rl_repo
provides the concourse (Bass) toolchain preinstalled in this container.
"""
import sys
import threading

import numpy as np

if '/opt/trn_rl_repo' not in sys.path:
    sys.path.insert(0, '/opt/trn_rl_repo')

# ---- static problem structure (hardcoded) ----
OFFS = [0, 65536, 131072, 196608]
N_ELEM = 196608
N_ROWS = 24576
LANE = 8
NUM_OPS = 65536
BATCH = 8
N_CORES = 8
RPP = 192            # addr rows per partition in the device layout
IMG_PAD = 66048      # 256 + 65536 + 256 elements per padded image

_rt = None           # lazily-built device runtime


def _stable_argsort_fast(k):
    """Exact stable argsort at introsort speed: unstable sort, then repair
    exact-tie runs by sorting their indices ascending."""
    p = np.argsort(k)
    ks = k[p]
    eq = np.nonzero(ks[1:] == ks[:-1])[0]
    if eq.size:
        breaks = np.nonzero(np.diff(eq) > 1)[0]
        starts = np.concatenate([eq[:1], eq[breaks + 1]])
        ends = np.concatenate([eq[breaks], eq[-1:]]) + 2
        for s0, e0 in zip(starts, ends):
            p[s0:e0] = np.sort(p[s0:e0])
    return p


def _build_bass(num_devices):
    """Construct + compile the per-core Bass program (see module docstring)."""
    from contextlib import ExitStack
    import concourse.bass as bass
    import concourse.tile as tile
    import concourse.bacc as bacc
    from concourse import mybir
    from concourse.masks import make_identity

    FP32 = mybir.dt.float32
    I32 = mybir.dt.int32
    ALU = mybir.AluOpType
    ACT = mybir.ActivationFunctionType

    nc = bacc.Bacc("TRN2", target_bir_lowering=False, debug=False,
                   enable_asserts=False, num_devices=num_devices,
                   num_swdge_queues=4)
    lo_d = nc.dram_tensor("perm_lo", [3 * IMG_PAD], mybir.dt.uint16,
                          kind="ExternalInput").ap()
    hi_d = nc.dram_tensor("perm_hi", [3 * IMG_PAD // 4], mybir.dt.uint8,
                          kind="ExternalInput").ap()
    par_d = nc.dram_tensor("params", [1408], FP32, kind="ExternalInput").ap()
    out_ap = nc.dram_tensor("pooled", [4, 64], FP32, kind="ExternalOutput").ap()

    with tile.TileContext(nc) as tc:
        with ExitStack() as ctx:
            io = ctx.enter_context(tc.tile_pool(name="io", bufs=1))
            imgp = ctx.enter_context(tc.tile_pool(name="imgp", bufs=2))
            featp = ctx.enter_context(tc.tile_pool(name="featp", bufs=3))
            bigp = ctx.enter_context(tc.tile_pool(name="bigp", bufs=1))
            psp = ctx.enter_context(tc.tile_pool(name="psp", bufs=2, space="PSUM"))
            psq = ctx.enter_context(tc.tile_pool(name="psq", bufs=2, space="PSUM"))
            accp = ctx.enter_context(tc.tile_pool(name="accp", bufs=2))
            dramp = ctx.enter_context(tc.tile_pool(name="dramp", bufs=1,
                                                   space="DRAM"))

            parb = io.tile([128, 1408], FP32)
            nc.sync.dma_start(parb[:],
                              bass.AP(par_d.tensor, 0, [[0, 128], [1, 1408]]))
            PMW, PMB, MCW, MCB = 0, 216, 240, 1392

            def w_ap(idx):
                return parb[:, idx:idx + 1]

            ident = io.tile([128, 128], FP32)
            make_identity(nc, ident[:])

            X = bigp.tile([128, 12416], FP32, name="X", tag="XM")
            I = bigp.tile([128, 1536], I32)

            # load + unpack the 3 padded images with row halo
            img = [None] * 3
            for m in range(3):
                lo_t = imgp.tile([128, 1024], mybir.dt.uint16, name=f"lo{m}")
                nc.sync.dma_start(lo_t[:], bass.AP(lo_d.tensor, m * IMG_PAD,
                                                   [[512, 128], [1, 1024]]))
                hi_t = imgp.tile([128, 256], mybir.dt.uint8, name=f"hi{m}")
                nc.sync.dma_start(hi_t[:],
                                  bass.AP(hi_d.tensor, m * (IMG_PAD // 4),
                                          [[128, 128], [1, 256]]))
                lo_f = imgp.tile([128, 1024], FP32, name=f"lof{m}")
                nc.vector.tensor_copy(lo_f[:], lo_t[:])
                hi_i = imgp.tile([128, 256], I32, name=f"hii{m}")
                nc.vector.tensor_copy(hi_i[:], hi_t[:])
                hi_f = imgp.tile([128, 1024], FP32, name=f"hif{m}")
                hi_j = imgp.tile([128, 256], I32, name=f"hij{m}")
                hf4 = hi_f[:].rearrange("p (c four) -> p c four", four=4)
                for j in range(4):
                    nc.vector.tensor_scalar(hi_j[:], hi_i[:], 2 * j, 3,
                                            ALU.logical_shift_right,
                                            ALU.bitwise_and)
                    nc.vector.tensor_copy(hf4[:, :, j], hi_j[:])
                im = imgp.tile([128, 1024], FP32, name=f"im{m}")
                nc.vector.scalar_tensor_tensor(im[:], hi_f[:], 65536.0, lo_f[:],
                                               ALU.mult, ALU.add)
                img[m] = im

            # scatter indices: I[p, 512m + 4p' + 2lr + cb] = perm of element
            # e = m*65536 + (2p'+lr)*256 + cb*128 + p
            I4 = I[:].rearrange("p (mm a four) -> p mm a four", mm=3, four=4)
            for m in range(3):
                for lr in range(2):
                    for cb in range(2):
                        ps = psp.tile([128, 128], FP32, space="PSUM")
                        base = 256 + lr * 256 + cb * 128
                        nc.tensor.transpose(ps[:], img[m][:, base:base + 128],
                                            ident[:])
                        nc.vector.tensor_copy(I4[:, m, :, 2 * lr + cb], ps[:])

            # feat conv + transpose into channel-interleaved X
            for m in range(3):
                for ch in range(8):
                    acc = featp.tile([128, 512], FP32, name="facc")
                    nc.scalar.activation(acc[:], img[m][:, 0:512], ACT.Identity,
                                         bias=w_ap(PMB + m * 8 + ch), scale=0.0)
                    for dr in range(3):
                        for dl in range(3):
                            w = w_ap(PMW + ((m * 8 + ch) * 3 + dr) * 3 + dl)
                            oc0, ic0 = (1, 0) if dl == 0 else (0, dl - 1)
                            nw = 255 if dl != 1 else 256
                            out_sl = acc[:].rearrange(
                                "p (two c) -> p two c", two=2)[:, :, oc0:oc0 + nw]
                            in_sl = img[m][:].rearrange(
                                "p (four c) -> p four c",
                                four=4)[:, dr:dr + 2, ic0:ic0 + nw]
                            nc.vector.scalar_tensor_tensor(
                                out_sl, in_sl, w, out_sl, ALU.mult, ALU.add)
                    nc.scalar.activation(acc[:], acc[:], ACT.Relu)
                    X4 = X[:, 0:12288].rearrange("p (mm pp f) -> p mm pp f",
                                                 mm=3, f=32)
                    for lr in range(2):
                        for cb in range(2):
                            ps = psp.tile([128, 128], FP32, space="PSUM")
                            nc.tensor.transpose(
                                ps[:], acc[:, lr * 256 + cb * 128:
                                           lr * 256 + cb * 128 + 128], ident[:])
                            nc.vector.tensor_copy(
                                X4[:, m, :, 16 * lr + 8 * cb + ch], ps[:])

            # +8 accounts for the zero-padded table head row
            nc.vector.tensor_scalar(I[:], I[:], 8, None, ALU.add)

            zpad = io.tile([1, 64], FP32)
            nc.gpsimd.memset(zpad[:], 0.0)

            # per-element indirect scatters: one 8ch row per partition/inst
            T = dramp.tile([N_ELEM + 16, 8], FP32, name="T")
            nc.sync.dma_start(bass.AP(T.tensor, 0, [[64, 1], [1, 64]]), zpad[:])
            nc.sync.dma_start(bass.AP(T.tensor, (N_ELEM + 8) * 8,
                                      [[64, 1], [1, 64]]), zpad[:])
            qnames = ["qPoolDynamic", "qPoolDynamic1", "qPoolDynamic2",
                      "qPoolDynamic3"]
            for k in range(1536):
                inst = nc.gpsimd.indirect_dma_start(
                    out=T[:, :],
                    out_offset=bass.IndirectOffsetOnAxis(ap=I[:, k:k + 1],
                                                         axis=0),
                    in_=X[:, 8 * k:8 * k + 8],
                    in_offset=None)
                inst.queue = qnames[k % 4]

            # halo readback: partition p covers addr rows 192p-1 .. 192p+193
            ms_ov = bigp.tile([128, 12416], FP32, name="ms_ov", tag="XM")
            nc.sync.dma_start(ms_ov[:],
                              bass.AP(T.tensor, 0, [[12288, 128], [1, 12416]]))

            blockmask = io.tile([128, 4], FP32)
            nc.gpsimd.memset(blockmask[:], 0.0)
            for bi in range(4):
                nc.gpsimd.memset(blockmask[bi * 32:(bi + 1) * 32, bi:bi + 1], 1.0)

            pooled = io.tile([4, 64], FP32)
            ms_p = ms_ov[:].ap[0]
            for co in range(16):
                acc = accp.tile([128, 1536], FP32, name="macc")
                acc3 = acc[:].rearrange("p (r l) -> p r l", l=LANE)
                nc.scalar.activation(acc[:], ms_ov[:, 0:1536], ACT.Identity,
                                     bias=w_ap(MCB + co), scale=0.0)
                for ci in range(8):
                    for dr in range(3):
                        for dl in range(3):
                            w = w_ap(MCW + ((co * 8 + ci) * 3 + dr) * 3 + dl)
                            ol0, il0 = (1, 0) if dl == 0 else (0, dl - 1)
                            nl = 7 if dl != 1 else 8
                            in_sl = bass.AP(ms_ov.tensor,
                                            dr * 64 + il0 * 8 + ci,
                                            [list(ms_p), [64, RPP], [8, nl]])
                            nc.vector.scalar_tensor_tensor(
                                acc3[:, :, ol0:ol0 + nl], in_sl,
                                w, acc3[:, :, ol0:ol0 + nl], ALU.mult, ALU.add)
                nc.scalar.activation(acc[:], acc[:], ACT.Relu)
                red = accp.tile([128, 4], FP32, name="red")
                for bj in range(4):
                    nc.vector.tensor_reduce(red[:, bj:bj + 1],
                                            acc3[:, :, 2 * bj:2 * bj + 2],
                                            mybir.AxisListType.XY, ALU.add)
                ps = psq.tile([4, 4], FP32, space="PSUM")
                nc.tensor.matmul(ps[:], blockmask[:], red[:], start=True,
                                 stop=True)
                nc.vector.tensor_copy(pooled[:, co * 4:co * 4 + 4], ps[:])

            nc.sync.dma_start(out_ap, pooled[:])
    nc.compile()
    return nc


class _Runtime:
    """Cached jitted SPMD executable + device-resident params."""

    def __init__(self):
        import jax
        from jax.sharding import Mesh, PartitionSpec, NamedSharding
        from jax.experimental.shard_map import shard_map
        from concourse.bass2jax import (install_neuronx_cc_hook, _bass_exec_p,
                                        partition_id_tensor)
        from concourse import mybir
        self.jax = jax
        self.devices = jax.devices()[:N_CORES]
        nc = _build_bass(N_CORES)
        install_neuronx_cc_hook()

        in_names, out_names, out_avals, zero_outs = [], [], [], []
        for alloc in nc.m.functions[0].allocations:
            if not isinstance(alloc, mybir.MemoryLocationSet):
                continue
            name = alloc.memorylocations[0].name
            if alloc.kind == "ExternalInput":
                if name != "partition_id":
                    in_names.append(name)
            elif alloc.kind == "ExternalOutput":
                out_names.append(name)
                shape = tuple(alloc.tensor_shape)
                dtype = mybir.dt.np(alloc.dtype)
                out_avals.append(jax.core.ShapedArray(shape, dtype))
                zero_outs.append(np.zeros(shape, dtype))
        all_names = in_names + out_names + ["partition_id"]
        self.in_names = in_names
        self.zero_outs = zero_outs
        n_params, n_outs = len(in_names), len(out_names)

        def _body(*args):
            ops = list(args) + [partition_id_tensor()]
            outs = _bass_exec_p.bind(
                *ops, out_avals=tuple(out_avals), in_names=tuple(all_names),
                out_names=tuple(out_names), lowering_input_output_aliases=(),
                sim_require_finite=True, sim_require_nnan=True, nc=nc)
            return tuple(outs)

        self.mesh = Mesh(np.asarray(self.devices), ("core",))
        self.sh = NamedSharding(self.mesh, PartitionSpec("core"))
        self.sharded = jax.jit(
            shard_map(_body, mesh=self.mesh,
                      in_specs=(PartitionSpec("core"),) * (n_params + n_outs),
                      out_specs=(PartitionSpec("core"),) * n_outs),
            donate_argnums=tuple(range(n_params, n_params + n_outs)),
            keep_unused=True)
        self._params_key = None
        self._params_g = None

    def params_global(self, params_np):
        key = params_np.tobytes()
        if self._params_key != key:
            parts = [self.jax.device_put(params_np, d) for d in self.devices]
            self._params_g = self.jax.make_array_from_single_device_arrays(
                (N_CORES * 1408,), self.sh, parts)
            self._params_key = key
        return self._params_g

    def assemble(self, parts):
        shape = (N_CORES * parts[0].shape[0],) + tuple(parts[0].shape[1:])
        return self.jax.make_array_from_single_device_arrays(
            shape, self.sh, parts)

    def dispatch(self, lo_parts, hi_parts, params_np):
        lo_g = self.assemble(lo_parts)
        hi_g = self.assemble(hi_parts)
        par_g = self.params_global(params_np)
        args = {"perm_lo": lo_g, "perm_hi": hi_g, "params": par_g}
        zeros = [np.zeros((N_CORES * z.shape[0],) + z.shape[1:], z.dtype)
                 for z in self.zero_outs]
        out = self.sharded(*[args[n] for n in self.in_names], *zeros)
        return out[0]


def _pack_perm(perm):
    """perm int32[196608] -> padded u16 lo + 2-bit-packed u8 hi."""
    padded = np.zeros(3 * IMG_PAD, np.int32)
    pv = padded.reshape(3, IMG_PAD)
    pv[:, 256:256 + 65536] = perm.reshape(3, 65536)
    lo = (padded & 0xFFFF).astype(np.uint16)
    hi4 = (padded >> 16).astype(np.uint8).reshape(-1, 4)
    hi = (hi4[:, 0] | (hi4[:, 1] << 2) | (hi4[:, 2] << 4)
          | (hi4[:, 3] << 6)).astype(np.uint8)
    return lo, hi


def _tier_small(h):
    return np.where(h <= 2, 1.0, np.where(h <= 4, 1.5,
           np.where(h <= 8, 2.0, 3.0))).astype(np.float32)


def _staged(v):
    """sum(fwd*tier(fwd) + bwd^2*tier(bwd)) per row; values are integer
    address deltas, almost all in tier 5 -- take the tier-5 base (BLAS dot
    for the squared half) and correct the few small-magnitude entries."""
    fwd = np.maximum(v, 0)
    bwd = fwd - v
    out = np.float32(5.0) * (fwd.sum(axis=-1, dtype=np.float32)
                             + np.einsum('ij,ij->i', bwd, bwd))
    bi, ei = np.nonzero((np.abs(v) < 17) & (v != 0))
    if bi.size:
        s = v[bi, ei]
        mag = np.abs(s)
        corr = (np.float32(5.0) - _tier_small(mag)) * np.where(s > 0, mag,
                                                               mag * mag)
        out -= np.bincount(bi, weights=corr, minlength=v.shape[0]
                           ).astype(np.float32)
    return out.astype(np.float32)


def kernel(mem_logits, gumbel_mem, gumbel_op, pm_conv_w, pm_conv_b,
           mem_conv_w, mem_conv_b, proj_w, proj_b):
    """Full (unsharded) inputs -> full (4, BATCH) float32 output."""
    global _rt
    import jax
    if _rt is None:
        _rt = _Runtime()

    mem_logits = np.asarray(mem_logits, dtype=np.float32)
    gumbel_mem = np.asarray(gumbel_mem, dtype=np.float32)
    gumbel_op = np.asarray(gumbel_op, dtype=np.float32)
    proj_w = np.asarray(proj_w, dtype=np.float32)
    proj_b = np.asarray(proj_b, dtype=np.float32)
    params = np.concatenate([
        np.asarray(pm_conv_w, np.float32).reshape(-1),
        np.asarray(pm_conv_b, np.float32).reshape(-1),
        np.asarray(mem_conv_w, np.float32).reshape(-1),
        np.asarray(mem_conv_b, np.float32).reshape(-1)]).astype(np.float32)

    # ---- memory permutations (host argsort), streamed per batch ----
    # The batched two-list device_put path dispatches in ~1 ms; the axon
    # client feeds the link from this thread's slack. A background thread
    # only handles the final result fetch, so its ~85 ms round-trip
    # overlaps the host math below.
    keys = mem_logits + gumbel_mem
    perm = np.empty((BATCH, N_ELEM), np.int32)
    lo_parts, hi_parts = [], []
    for b in range(BATCH):
        perm[b] = _stable_argsort_fast(keys[b])
        lo, hi = _pack_perm(perm[b])
        d = _rt.devices[b]
        lo_b, hi_b = jax.device_put([lo, hi], [d, d])
        lo_parts.append(lo_b)
        hi_parts.append(hi_b)

    fut = _rt.dispatch(lo_parts, hi_parts, params)
    result = {}

    def _fetch():
        result["pooled"] = np.asarray(fut)

    th = threading.Thread(target=_fetch)
    th.start()

    # ---- host work overlapped with the device leg ----
    mem_lp = np.empty((BATCH,), np.float32)
    for b in range(BATCH):
        s = mem_logits[b][perm[b]]
        m = s[-1]
        ssum = s.sum(dtype=np.float32)
        np.subtract(s, m, out=s)
        np.exp(s, out=s)
        suf = np.cumsum(s[::-1], dtype=np.float32)[::-1]
        np.log(suf, out=suf)
        mem_lp[b] = (ssum - (suf.sum(dtype=np.float32)
                             + np.float32(N_ELEM) * m))
    A = perm[:, OFFS[0]:OFFS[1]].astype(np.float32)
    Bc = perm[:, OFFS[1]:OFFS[2]].astype(np.float32)
    Cc = perm[:, OFFS[2]:OFFS[3]].astype(np.float32)
    intra_pen = (_staged(Bc - A) + _staged(Cc - Bc)).astype(np.float32)

    th.join()
    dev = result["pooled"].reshape(BATCH, 4, 16, 4)       # [b, bi, co, bj]
    pooled = (dev.transpose(0, 2, 1, 3).reshape(BATCH, 256)
              * np.float32(1.0 / 12288.0))

    # ---- projection + op path (host) ----
    opl = (proj_w @ pooled.T).T
    opl += proj_b
    opl = opl.astype(np.float32)
    op_lp = np.empty((BATCH,), np.float32)
    inter_d = np.empty((BATCH, NUM_OPS - 1), np.float32)
    for b in range(BATCH):
        o = _stable_argsort_fast(opl[b] + gumbel_op[b])
        so = opl[b][o]
        m = so[-1]
        ssum = so.sum(dtype=np.float32)
        np.subtract(so, m, out=so)
        np.exp(so, out=so)
        suf = np.cumsum(so[::-1], dtype=np.float32)[::-1]
        np.log(suf, out=suf)
        op_lp[b] = (ssum - (suf.sum(dtype=np.float32)
                            + np.float32(NUM_OPS) * m))
        Ao = A[b][o]
        Co = Cc[b][o]
        inter_d[b] = Ao[1:] - Co[:-1]
    inter_pen = _staged(inter_d)

    out = np.stack([inter_pen, intra_pen, op_lp, mem_lp]).astype(np.float32)
    return np.ascontiguousarray(out)
